# revision 40
# baseline (speedup 1.0000x reference)
"""Trainium2 Bass kernel for nn_CachedCompressedLinear.

out[16, 11008] = x[16, 4096] @ ((w_q - 128) * scale).T + bias

Sharding: column-parallel over 8 NeuronCores; each core owns a 1376-wide
slice of out_features (8 * 1376 = 11008).

The weights are dequantized ON THE HOST directly to fp8 e3m4 at 8x scale
(w8 = fp8e3((c - 128) * s * 8)), so HBM traffic stays at 1 byte/element
(5.64 MB/core) and there is NO on-device decode at all: the PE consumes
the fp8 tiles directly as the moving operand against a bf16 stationary x
(mixed-dtype matmul, verified bit-exact on HW).  e3m4's 4 mantissa bits
give a 1.41e-2 relative error against the 2e-2 budget (e4m3 would be
2.7e-2).  The fp8 pre-scale is a power of two, so each PSUM chunk's
epilogue is one DVE scalar_tensor_tensor (psum * 1/8 + bias) against a
host-broadcast bias, followed by its own HWDGE out-DMA.

All weights stream k-ordered on the gpsimd SWDGE ring, small groups
first for fast first-tile completion (each dma_start costs ~650ns of
issue time, so the bulk rides 4-tile groups).  x is split into separate
head/tail TILES (the tile dep-tracker is tile-granular, so k<8 matmuls
must not wait on the tail transfer); bias rides the idle scalar ring.
Dummy matmuls on a memset tile warm the PE's HAM clock gate (1.2 ->
2.4 GHz after ~3.4us of sustained activity) and a graded weave of warm
matmuls between the early k-tiles keeps the activity window alive
through the DMA cold-ramp; the last 3 k-tiles run chunk-contiguously so
the epilogues overlap the closing matmuls.
"""

import sys

if "/opt/trn_rl_repo" not in sys.path:
    sys.path.insert(0, "/opt/trn_rl_repo")

import numpy as np
import ml_dtypes

IN_F = 4096
OUT_F = 11008
BATCH = 16
N_CORES = 8
O_PER = 1376  # out_features per core
K_TILES = IN_F // 128  # 32
M = 16  # stationary columns: x in bf16
ALPHA = 8.0  # fp8 pre-scale (power of two -> exact epilogue)
INV_ALPHA = 1.0 / ALPHA
CHUNKS = [(0, 512), (512, 512), (1024, 352)]
# weight groups as (k0, count, ring): ALL weights on the gpsimd SWDGE ring
# in k-order — splitting across rings slows everything down (the rings
# share the 16 SDMA engines and each ring's ramp restarts); each
# dma_start costs ~650ns of gpsimd issue time, so small groups only at
# the head (fast first completion) and tail
W_GROUPS = [
    (1, 2, "gpsimd"),
    (3, 2, "gpsimd"),
    (5, 2, "gpsimd"),
    (7, 2, "gpsimd"),
    (9, 2, "gpsimd"),
    (11, 2, "gpsimd"),
    (13, 4, "gpsimd"),
    (17, 4, "gpsimd"),
    (21, 4, "gpsimd"),
    (25, 4, "gpsimd"),
    (29, 3, "gpsimd"),
]
BIAS1_K = 16  # k-tile after which chunk1's bias matmul is folded in
X_SPLIT = 4  # first X_SPLIT k-tiles of x go in their own tile + DMA
N_WARM = 18  # upfront dummy matmuls: span a FULL ~3.4us HAM window at the
# cold clock (18 x ~213ns) so the PE is already at 2.4 GHz when k0 lands
# graded warm-matmul weave: k-tile -> count (bridges the DMA cold ramp)
WARM_WEAVE = {1: 3, 2: 3, 3: 2, 4: 2, 5: 2, 6: 1, 7: 1, 8: 1}
WARM_N = 256  # moving width of each warm matmul

_BUILT = None


def _build():
    """Build the (SPMD, per-core) Bass program once."""
    import concourse.bass as bass
    import concourse.tile as tile
    from concourse import bacc, mybir

    dt = mybir.dt
    nc = bacc.Bacc("TRN2", target_bir_lowering=False, debug=False)

    wt8 = nc.dram_tensor("wt8", [128, K_TILES * O_PER], dt.float8e3,
                         kind="ExternalInput")
    xt2 = nc.dram_tensor(
        "xt2", [128, K_TILES * M], dt.bfloat16, kind="ExternalInput"
    )
    bias16 = nc.dram_tensor(
        "bias16", [BATCH, O_PER], dt.bfloat16, kind="ExternalInput"
    )
    bias8 = nc.dram_tensor(
        "bias8", [1, O_PER], dt.bfloat16, kind="ExternalInput"
    )
    out = nc.dram_tensor("out", [BATCH, O_PER], dt.float32, kind="ExternalOutput")

    with tile.TileContext(nc) as tc:
        with (
            tc.tile_pool(name="consts", bufs=1) as consts,
            tc.tile_pool(name="w8", bufs=1) as w8p,
            tc.tile_pool(name="psum", bufs=1, space=bass.MemorySpace.PSUM) as psump,
            tc.tile_pool(name="outp", bufs=1) as outp,
        ):
            # warm the PE clock gate with dummy matmuls on a memset tile;
            # the memset rides the otherwise-idle DVE so gpsimd's queue
            # (which issues the weight DMAs) is not delayed and PE
            # activity starts as early as possible.
            warm = consts.tile([128, WARM_N], dt.bfloat16, name="warm")
            nc.vector.memset(warm[:], 1.0)
            ps_warm = psump.tile([M, WARM_N], dt.float32, name="psw", tag="psw")
            for _ in range(N_WARM):
                nc.tensor.matmul(ps_warm[:], warm[:, 0:M], warm[:],
                                 start=True, stop=True)

            # Weights spread across all four DMA rings so each ring's
            # cold-start bandwidth ramp overlaps; k-order is preserved per
            # ring and the PE consumes in k-order via per-tile sems.  The
            # first X_SPLIT k-tiles of x lead on the sync ring; the rest of
            # x and bias are interleaved behind the sync-ring weights.
            rings = {"gpsimd": nc.gpsimd, "sync": nc.sync,
                     "scalar": nc.scalar}
            # x head and tail are SEPARATE tiles: the tile dep-tracker is
            # tile-granular, so matmuls for k < X_SPLIT must not be gated
            # on the slower x-tail transfer.  bias (bf16, needed only at
            # the epilogue) rides the otherwise-idle scalar ring.
            x_head = consts.tile([128, X_SPLIT * M], dt.bfloat16,
                                 name="x_head")
            nc.sync.dma_start(x_head[:], xt2[:, 0:X_SPLIT * M])
            # k0's chunk-0 columns ride the sync HWDGE ring: cold it only
            # manages ~70 GB/s, but its ~0.6us receipt beats SWDGE's
            # ~1.3us, so the PE's first real matmul starts ~1us earlier.
            w0, ww0 = CHUNKS[0]
            wt_0a = w8p.tile([128, 1, ww0], dt.float8e3, tag="w8_0a")
            nc.sync.dma_start(wt_0a[:, 0:1, :], wt8[:, w0:w0 + ww0])
            x_tail = consts.tile([128, (K_TILES - X_SPLIT) * M], dt.bfloat16,
                                 name="x_tail")
            nc.sync.dma_start(x_tail[:], xt2[:, X_SPLIT * M:])
            bias_sb = consts.tile([BATCH, O_PER], dt.bfloat16)
            nc.scalar.dma_start(bias_sb[:], bias16[:])
            bias8_sb = consts.tile([1, O_PER], dt.bfloat16)
            nc.scalar.dma_start(bias8_sb[:], bias8[:])
            # k0's chunk-1/2 columns lead the gpsimd SWDGE stream
            wt_0b = w8p.tile([128, 1, O_PER - ww0], dt.float8e3, tag="w8_0b")
            nc.gpsimd.dma_start(wt_0b[:, 0:1, :], wt8[:, ww0:O_PER])

            def x_blk(k):
                if k < X_SPLIT:
                    return x_head[:, k * M:(k + 1) * M]
                return x_tail[:, (k - X_SPLIT) * M:(k - X_SPLIT + 1) * M]

            w_tiles = []
            for gi, (k0, G, ring) in enumerate(W_GROUPS):
                wt_t = w8p.tile([128, G, O_PER], dt.float8e3, tag=f"w8_{gi}")
                rings[ring].dma_start(
                    wt_t[:, 0:G, :],
                    wt8[:, k0 * O_PER:(k0 + G) * O_PER],
                )
                w_tiles.append((k0, G, wt_t))

            psums = [
                psump.tile([M, w], dt.float32, name=f"ps{i}", tag=f"ps{i}")
                for i, (_, w) in enumerate(CHUNKS)
            ]

            # out rings: HWDGE only (short receipt; rings are warm by now)
            out_rings = [nc.sync, nc.scalar, nc.sync]
            alu = mybir.AluOpType

            # epilogue: DVE scalar_tensor_tensor (psum * 1/8 + bias) for
            # chunks 0/2; chunk 1 runs on ACT (scale only — its bias was
            # already folded into PSUM by a K=1 matmul) so the two engines
            # drain the three chunks in parallel at the tail
            def epilogue(i, o, w):
                comb = outp.tile([BATCH, w], dt.float32, name=f"comb{i}")
                if i == 1:
                    nc.scalar.activation(
                        comb[:], psums[i][0:BATCH, :],
                        mybir.ActivationFunctionType.Identity,
                        scale=INV_ALPHA)
                else:
                    nc.vector.scalar_tensor_tensor(
                        comb[:], psums[i][0:BATCH, :], INV_ALPHA,
                        bias_sb[:, o:o + w], alu.mult, alu.add)
                out_rings[i].dma_start(out[:][:, o:o + w], comb[:])

            # map k -> (t, wt_t) for the tail reordering
            k_tile = {}
            for k0, G, wt_t in w_tiles:
                for t in range(G):
                    k_tile[k0 + t] = (t, wt_t)

            def mv_of(k, i):
                o, w = CHUNKS[i]
                if k == 0:
                    if i == 0:
                        return wt_0a[:, 0, 0:w]
                    return wt_0b[:, 0, o - ww0:o - ww0 + w]
                t, wt_t = k_tile[k]
                return wt_t[:, t, o:o + w]

            TAIL = 3  # last TAIL k-tiles run chunk-contiguously
            for k in range(K_TILES - TAIL):
                for i in range(len(CHUNKS)):
                    nc.tensor.matmul(
                        psums[i][:, :],
                        x_blk(k),
                        mv_of(k, i),
                        start=(k == 0),
                        stop=False,
                    )
                if k == BIAS1_K:
                    # fold chunk1's bias into PSUM: K=1 matmul of the
                    # all-ones warm row against the 8x-bias row
                    o1, w1 = CHUNKS[1]
                    nc.tensor.matmul(
                        psums[1][:, :],
                        warm[0:1, 0:M],
                        bias8_sb[0:1, o1:o1 + w1],
                        start=False,
                        stop=False,
                    )
                for _ in range(WARM_WEAVE.get(k, 0)):
                    # keep the PE's HAM activity window alive through
                    # early DMA-ramp micro-stalls
                    nc.tensor.matmul(ps_warm[:], warm[:, 0:M],
                                     warm[:], start=True, stop=True)

            # tail: all weights are resident by now, so run the last TAIL
            # k-tiles chunk-contiguously — each chunk closes while the PE
            # still streams the next chunk, overlapping the DVE epilogues
            # with the remaining matmuls
            for i, (o, w) in enumerate(CHUNKS):
                for k in range(K_TILES - TAIL, K_TILES):
                    nc.tensor.matmul(
                        psums[i][:, :],
                        x_blk(k),
                        mv_of(k, i),
                        start=False,
                        stop=(k == K_TILES - 1),
                    )
                epilogue(i, o, w)

    nc.compile()
    return nc


def _get_built():
    global _BUILT
    if _BUILT is None:
        _BUILT = _build()
    return _BUILT


def make_in_maps(x, w_q, scale, bias):
    """Host-side shard + layout prep. Returns per-core input dicts."""
    x = np.asarray(x, dtype=np.float32)
    w_q = np.asarray(w_q, dtype=np.int32)
    scale = np.asarray(scale, dtype=np.float32)
    bias = np.asarray(bias, dtype=np.float32)
    s = float(scale.reshape(-1)[0])

    xT = np.ascontiguousarray(x.T)  # [4096, 16]
    x16 = xT.astype(ml_dtypes.bfloat16)
    # prepack to the SBUF layout [128, K_TILES*M]: partition p holds,
    # for each k-tile t, the stationary block row (t*128 + p)
    xt2 = np.ascontiguousarray(
        x16.reshape(K_TILES, 128, M).transpose(1, 0, 2).reshape(128, K_TILES * M)
    )

    in_maps = []
    for c in range(N_CORES):
        # fp8 e3m4 dequantized weights at ALPHA x scale, transposed to
        # [4096, 1376] then packed so partition p holds, for k-tile t,
        # row (t*128 + p): [128, 32*1376]
        wt_c = w_q[c * O_PER:(c + 1) * O_PER].T.astype(np.float32)
        w8_c = ((wt_c - 128.0) * (s * ALPHA)).astype(ml_dtypes.float8_e3m4)
        wt8_c = np.ascontiguousarray(
            w8_c.reshape(K_TILES, 128, O_PER)
            .transpose(1, 0, 2)
            .reshape(128, K_TILES * O_PER)
        )
        # bias broadcast to all 16 batch rows (added in the DVE epilogue)
        b16 = np.ascontiguousarray(
            np.broadcast_to(bias[c * O_PER:(c + 1) * O_PER], (BATCH, O_PER))
        ).astype(ml_dtypes.bfloat16)
        # 8x bias row for chunk1's in-PSUM bias matmul
        b8 = (bias[c * O_PER:(c + 1) * O_PER] * ALPHA).astype(
            ml_dtypes.bfloat16).reshape(1, O_PER)
        in_maps.append({"wt8": wt8_c, "xt2": xt2, "bias16": b16,
                        "bias8": np.ascontiguousarray(b8)})
    return in_maps


def run(inputs, trace=False):
    """Run on the 8 NeuronCores. Returns (full_output, BassKernelResults)."""
    from concourse.bass_utils import run_bass_kernel_spmd

    in_maps = make_in_maps(**inputs)
    nc = _get_built()
    res = run_bass_kernel_spmd(nc, in_maps, list(range(N_CORES)), trace=trace)
    parts = [np.asarray(res.results[c]["out"]) for c in range(N_CORES)]
    full = np.concatenate(parts, axis=1)[:, :OUT_F].astype(np.float32)
    return full, res


def kernel(**inputs) -> np.ndarray:
    full, _ = run(inputs, trace=False)
    return full


# revision 41
# speedup vs baseline: 1.0110x; 1.0110x over previous
"""Trainium2 Bass kernel for nn_CachedCompressedLinear.

out[16, 11008] = x[16, 4096] @ ((w_q - 128) * scale).T + bias

Sharding: column-parallel over 8 NeuronCores; each core owns a 1376-wide
slice of out_features (8 * 1376 = 11008).

The weights are dequantized ON THE HOST directly to fp8 e3m4 at 8x scale
(w8 = fp8e3((c - 128) * s * 8)), so HBM traffic stays at 1 byte/element
(5.64 MB/core) and there is NO on-device decode at all: the PE consumes
the fp8 tiles directly as the moving operand against a bf16 stationary x
(mixed-dtype matmul, verified bit-exact on HW).  e3m4's 4 mantissa bits
give a 1.41e-2 relative error against the 2e-2 budget (e4m3 would be
2.7e-2).  The fp8 pre-scale is a power of two, so each PSUM chunk's
epilogue is one DVE scalar_tensor_tensor (psum * 1/8 + bias) against a
host-broadcast bias, followed by its own HWDGE out-DMA.

All weights stream k-ordered on the gpsimd SWDGE ring, small groups
first for fast first-tile completion (each dma_start costs ~650ns of
issue time, so the bulk rides 4-tile groups).  x is split into separate
head/tail TILES (the tile dep-tracker is tile-granular, so k<8 matmuls
must not wait on the tail transfer); bias rides the idle scalar ring.
Dummy matmuls on a memset tile warm the PE's HAM clock gate (1.2 ->
2.4 GHz after ~3.4us of sustained activity) and a graded weave of warm
matmuls between the early k-tiles keeps the activity window alive
through the DMA cold-ramp; the last 3 k-tiles run chunk-contiguously so
the epilogues overlap the closing matmuls.
"""

import sys

if "/opt/trn_rl_repo" not in sys.path:
    sys.path.insert(0, "/opt/trn_rl_repo")

import numpy as np
import ml_dtypes

IN_F = 4096
OUT_F = 11008
BATCH = 16
N_CORES = 8
O_PER = 1376  # out_features per core
K_TILES = IN_F // 128  # 32
M = 16  # stationary columns: x in bf16
ALPHA = 8.0  # fp8 pre-scale (power of two -> exact epilogue)
INV_ALPHA = 1.0 / ALPHA
CHUNKS = [(0, 512), (512, 512), (1024, 352)]
# weight groups as (k0, count, ring): ALL weights on the gpsimd SWDGE ring
# in k-order — splitting across rings slows everything down (the rings
# share the 16 SDMA engines and each ring's ramp restarts); each
# dma_start costs ~650ns of gpsimd issue time, so small groups only at
# the head (fast first completion) and tail
W_GROUPS = [
    (1, 2, "gpsimd"),
    (3, 2, "gpsimd"),
    (5, 4, "gpsimd"),
    (9, 4, "gpsimd"),
    (13, 4, "gpsimd"),
    (17, 4, "gpsimd"),
    (21, 4, "gpsimd"),
    (25, 4, "gpsimd"),
    (29, 3, "gpsimd"),
]
BIAS1_K = 16  # k-tile after which chunk1's bias matmul is folded in
X_SPLIT = 8  # first X_SPLIT k-tiles of x go in their own tile + DMA
N_WARM = 18  # upfront dummy matmuls: span a FULL ~3.4us HAM window at the
# cold clock (18 x ~213ns) so the PE is already at 2.4 GHz when k0 lands
# graded warm-matmul weave: k-tile -> count (bridges the DMA cold ramp)
WARM_WEAVE = {1: 3, 2: 3, 3: 2, 4: 2, 5: 2, 6: 1, 7: 1, 8: 1}
WARM_N = 256  # moving width of each warm matmul

_BUILT = None


def _build():
    """Build the (SPMD, per-core) Bass program once."""
    import concourse.bass as bass
    import concourse.tile as tile
    from concourse import bacc, mybir

    dt = mybir.dt
    nc = bacc.Bacc("TRN2", target_bir_lowering=False, debug=False)

    wt8 = nc.dram_tensor("wt8", [128, K_TILES * O_PER], dt.float8e3,
                         kind="ExternalInput")
    xt2 = nc.dram_tensor(
        "xt2", [128, K_TILES * M], dt.bfloat16, kind="ExternalInput"
    )
    bias16 = nc.dram_tensor(
        "bias16", [BATCH, O_PER], dt.bfloat16, kind="ExternalInput"
    )
    bias8 = nc.dram_tensor(
        "bias8", [1, O_PER], dt.bfloat16, kind="ExternalInput"
    )
    out = nc.dram_tensor("out", [BATCH, O_PER], dt.float32, kind="ExternalOutput")

    with tile.TileContext(nc) as tc:
        with (
            tc.tile_pool(name="consts", bufs=1) as consts,
            tc.tile_pool(name="w8", bufs=1) as w8p,
            tc.tile_pool(name="psum", bufs=1, space=bass.MemorySpace.PSUM) as psump,
            tc.tile_pool(name="outp", bufs=1) as outp,
        ):
            # warm the PE clock gate with dummy matmuls on a memset tile;
            # the memset rides the otherwise-idle DVE so gpsimd's queue
            # (which issues the weight DMAs) is not delayed and PE
            # activity starts as early as possible.
            warm = consts.tile([128, WARM_N], dt.bfloat16, name="warm")
            nc.vector.memset(warm[:], 1.0)
            ps_warm = psump.tile([M, WARM_N], dt.float32, name="psw", tag="psw")
            for _ in range(N_WARM):
                nc.tensor.matmul(ps_warm[:], warm[:, 0:M], warm[:],
                                 start=True, stop=True)

            # Weights spread across all four DMA rings so each ring's
            # cold-start bandwidth ramp overlaps; k-order is preserved per
            # ring and the PE consumes in k-order via per-tile sems.  The
            # first X_SPLIT k-tiles of x lead on the sync ring; the rest of
            # x and bias are interleaved behind the sync-ring weights.
            rings = {"gpsimd": nc.gpsimd, "sync": nc.sync,
                     "scalar": nc.scalar}
            # x head and tail are SEPARATE tiles: the tile dep-tracker is
            # tile-granular, so matmuls for k < X_SPLIT must not be gated
            # on the slower x-tail transfer.  bias (bf16, needed only at
            # the epilogue) rides the otherwise-idle scalar ring.
            x_head = consts.tile([128, X_SPLIT * M], dt.bfloat16,
                                 name="x_head")
            nc.sync.dma_start(x_head[:], xt2[:, 0:X_SPLIT * M])
            # k0's chunk-0 columns ride the sync HWDGE ring: cold it only
            # manages ~70 GB/s, but its ~0.6us receipt beats SWDGE's
            # ~1.3us, so the PE's first real matmul starts ~1us earlier.
            w0, ww0 = CHUNKS[0]
            wt_0a = w8p.tile([128, 1, ww0], dt.float8e3, tag="w8_0a")
            nc.sync.dma_start(wt_0a[:, 0:1, :], wt8[:, w0:w0 + ww0])
            x_tail = consts.tile([128, (K_TILES - X_SPLIT) * M], dt.bfloat16,
                                 name="x_tail")
            nc.sync.dma_start(x_tail[:], xt2[:, X_SPLIT * M:])
            bias_sb = consts.tile([BATCH, O_PER], dt.bfloat16)
            nc.scalar.dma_start(bias_sb[:], bias16[:])
            bias8_sb = consts.tile([1, O_PER], dt.bfloat16)
            nc.scalar.dma_start(bias8_sb[:], bias8[:])
            # k0's chunk-1/2 columns lead the gpsimd SWDGE stream
            wt_0b = w8p.tile([128, 1, O_PER - ww0], dt.float8e3, tag="w8_0b")
            nc.gpsimd.dma_start(wt_0b[:, 0:1, :], wt8[:, ww0:O_PER])

            def x_blk(k):
                if k < X_SPLIT:
                    return x_head[:, k * M:(k + 1) * M]
                return x_tail[:, (k - X_SPLIT) * M:(k - X_SPLIT + 1) * M]

            w_tiles = []
            for gi, (k0, G, ring) in enumerate(W_GROUPS):
                wt_t = w8p.tile([128, G, O_PER], dt.float8e3, tag=f"w8_{gi}")
                rings[ring].dma_start(
                    wt_t[:, 0:G, :],
                    wt8[:, k0 * O_PER:(k0 + G) * O_PER],
                )
                w_tiles.append((k0, G, wt_t))

            psums = [
                psump.tile([M, w], dt.float32, name=f"ps{i}", tag=f"ps{i}")
                for i, (_, w) in enumerate(CHUNKS)
            ]

            # out rings: HWDGE only (short receipt; rings are warm by now)
            out_rings = [nc.sync, nc.scalar, nc.sync]
            alu = mybir.AluOpType

            # epilogue: DVE scalar_tensor_tensor (psum * 1/8 + bias) for
            # chunks 0/2; chunk 1 runs on ACT (scale only — its bias was
            # already folded into PSUM by a K=1 matmul) so the two engines
            # drain the three chunks in parallel at the tail
            def epilogue(i, o, w):
                comb = outp.tile([BATCH, w], dt.float32, name=f"comb{i}")
                if i == 1:
                    nc.scalar.activation(
                        comb[:], psums[i][0:BATCH, :],
                        mybir.ActivationFunctionType.Identity,
                        scale=INV_ALPHA)
                else:
                    nc.vector.scalar_tensor_tensor(
                        comb[:], psums[i][0:BATCH, :], INV_ALPHA,
                        bias_sb[:, o:o + w], alu.mult, alu.add)
                out_rings[i].dma_start(out[:][:, o:o + w], comb[:])

            # map k -> (t, wt_t) for the tail reordering
            k_tile = {}
            for k0, G, wt_t in w_tiles:
                for t in range(G):
                    k_tile[k0 + t] = (t, wt_t)

            def mv_of(k, i):
                o, w = CHUNKS[i]
                if k == 0:
                    if i == 0:
                        return wt_0a[:, 0, 0:w]
                    return wt_0b[:, 0, o - ww0:o - ww0 + w]
                t, wt_t = k_tile[k]
                return wt_t[:, t, o:o + w]

            TAIL = 3  # last TAIL k-tiles run chunk-contiguously
            for k in range(K_TILES - TAIL):
                for i in range(len(CHUNKS)):
                    nc.tensor.matmul(
                        psums[i][:, :],
                        x_blk(k),
                        mv_of(k, i),
                        start=(k == 0),
                        stop=False,
                    )
                if k == BIAS1_K:
                    # fold chunk1's bias into PSUM: K=1 matmul of the
                    # all-ones warm row against the 8x-bias row
                    o1, w1 = CHUNKS[1]
                    nc.tensor.matmul(
                        psums[1][:, :],
                        warm[0:1, 0:M],
                        bias8_sb[0:1, o1:o1 + w1],
                        start=False,
                        stop=False,
                    )
                for _ in range(WARM_WEAVE.get(k, 0)):
                    # keep the PE's HAM activity window alive through
                    # early DMA-ramp micro-stalls
                    nc.tensor.matmul(ps_warm[:], warm[:, 0:M],
                                     warm[:], start=True, stop=True)

            # tail: all weights are resident by now, so run the last TAIL
            # k-tiles chunk-contiguously — each chunk closes while the PE
            # still streams the next chunk, overlapping the DVE epilogues
            # with the remaining matmuls
            for i, (o, w) in enumerate(CHUNKS):
                for k in range(K_TILES - TAIL, K_TILES):
                    nc.tensor.matmul(
                        psums[i][:, :],
                        x_blk(k),
                        mv_of(k, i),
                        start=False,
                        stop=(k == K_TILES - 1),
                    )
                epilogue(i, o, w)

    nc.compile()
    return nc


def _get_built():
    global _BUILT
    if _BUILT is None:
        _BUILT = _build()
    return _BUILT


def make_in_maps(x, w_q, scale, bias):
    """Host-side shard + layout prep. Returns per-core input dicts."""
    x = np.asarray(x, dtype=np.float32)
    w_q = np.asarray(w_q, dtype=np.int32)
    scale = np.asarray(scale, dtype=np.float32)
    bias = np.asarray(bias, dtype=np.float32)
    s = float(scale.reshape(-1)[0])

    xT = np.ascontiguousarray(x.T)  # [4096, 16]
    x16 = xT.astype(ml_dtypes.bfloat16)
    # prepack to the SBUF layout [128, K_TILES*M]: partition p holds,
    # for each k-tile t, the stationary block row (t*128 + p)
    xt2 = np.ascontiguousarray(
        x16.reshape(K_TILES, 128, M).transpose(1, 0, 2).reshape(128, K_TILES * M)
    )

    in_maps = []
    for c in range(N_CORES):
        # fp8 e3m4 dequantized weights at ALPHA x scale, transposed to
        # [4096, 1376] then packed so partition p holds, for k-tile t,
        # row (t*128 + p): [128, 32*1376]
        wt_c = w_q[c * O_PER:(c + 1) * O_PER].T.astype(np.float32)
        w8_c = ((wt_c - 128.0) * (s * ALPHA)).astype(ml_dtypes.float8_e3m4)
        wt8_c = np.ascontiguousarray(
            w8_c.reshape(K_TILES, 128, O_PER)
            .transpose(1, 0, 2)
            .reshape(128, K_TILES * O_PER)
        )
        # bias broadcast to all 16 batch rows (added in the DVE epilogue)
        b16 = np.ascontiguousarray(
            np.broadcast_to(bias[c * O_PER:(c + 1) * O_PER], (BATCH, O_PER))
        ).astype(ml_dtypes.bfloat16)
        # 8x bias row for chunk1's in-PSUM bias matmul
        b8 = (bias[c * O_PER:(c + 1) * O_PER] * ALPHA).astype(
            ml_dtypes.bfloat16).reshape(1, O_PER)
        in_maps.append({"wt8": wt8_c, "xt2": xt2, "bias16": b16,
                        "bias8": np.ascontiguousarray(b8)})
    return in_maps


def run(inputs, trace=False):
    """Run on the 8 NeuronCores. Returns (full_output, BassKernelResults)."""
    from concourse.bass_utils import run_bass_kernel_spmd

    in_maps = make_in_maps(**inputs)
    nc = _get_built()
    res = run_bass_kernel_spmd(nc, in_maps, list(range(N_CORES)), trace=trace)
    parts = [np.asarray(res.results[c]["out"]) for c in range(N_CORES)]
    full = np.concatenate(parts, axis=1)[:, :OUT_F].astype(np.float32)
    return full, res


def kernel(**inputs) -> np.ndarray:
    full, _ = run(inputs, trace=False)
    return full
